# revision 38
# baseline (speedup 1.0000x reference)
"""Distributed GQA attention layer (seq=2048, dim=4096, 32 q heads / 8 kv heads,
rope theta=5e5, causal) on 8 TRN2 NeuronCores.

Sharding: tensor-parallel over heads. Core c owns q heads 4c..4c+3 and kv head c.
Each core computes its 4 heads' attention output in transposed layout
attnT_local [512, 2048], an AllGather over the partition axis assembles
attnT_full [4096, 2048], and each core then computes 512 output columns of the
final projection: out_c [2048, 512] = attnT_full.T @ woT_c.  The host
concatenates the 8 column blocks.

v2 structure: one fused loop over the 4 seq chunks -- projections(j),
attention(j), AllGather(j), out-projection(j-2) -- so collectives start as
early as possible and the out-projection pipeline hides them.  DMA traffic is
split between the sync HWDGE queue (x tiles, agin) and the gpsimd SWDGE queue
(weights, agout pulls, output writes) with large descriptors.

Device-side layout tricks (all host-prepped):
 - x fed transposed [dim, seq]; all weights fed as [dim(contract), out].
 - rope pairs (2i, 2i+1) are permuted to half-split form by permuting wq/wk
   rows, making rope a "rotate-half": r = t*CH + swap_halves(t)*SH, where
   swap_halves is a 128x128 permutation matmul and CH/SH are [128, seq]
   tables. The q-k inner product is invariant under the shared permutation.
 - 1/sqrt(hd) folded into wq.
 - softmax runs unnormalized; a ones-column appended to v makes the PV matmul
   accumulate the denominator in psum column 128, and the normalization is
   fused into the per-q-row scale before the transpose.
 - v projected weight-stationary into [hd, seq] then PE-transposed to
   [seq, hd] tiles.
PSUM budget (8 banks): accp 3 (q-projection 2-head passes + k/v acc; also the
wo accumulators in 2-sq passes), ps512 2 (rope swap product + score tiles),
apsp 2 (PV accumulators, 2x129 cols packed per bank), pstr 1 (transposes).
"""
import sys

sys.path.insert(0, "/opt/trn_rl_repo")
import numpy as np

import concourse.bass as bass
import concourse.mybir as mybir
import concourse.tile as tile
from concourse import bacc
from concourse.bass_utils import run_bass_kernel_spmd

SEQ = 2048
DIM = 4096
NH, NKV, HD = 32, 8, 128
THETA = 500000.0
NCORE = 8
HPC = NH // NCORE          # 4 q heads per core
HALF = HD // 2
SCALE = 1.0 / np.sqrt(HD)
NDT = DIM // 128           # contraction tiles for qkv projections (32)
NET = (NH * HD) // 128     # e-dim tiles for the output projection (32)
NCH = SEQ // 512           # 4 seq chunks
XB = 8                     # x d-tiles per DMA block
NXB = NDT // XB            # 4 x-blocks per chunk
F32 = mybir.dt.float32
AF = mybir.ActivationFunctionType
ALU = mybir.AluOpType
CD = mybir.dt.bfloat16


def build():
    nc = bacc.Bacc("TRN2", target_bir_lowering=False, debug=False, num_devices=NCORE)
    x_e = nc.dram_tensor("x", [DIM, SEQ], CD, kind="ExternalInput")
    wq_e = nc.dram_tensor("wq", [DIM, HPC * HD], CD, kind="ExternalInput")
    wk_e = nc.dram_tensor("wk", [DIM, HD], CD, kind="ExternalInput")
    wv_e = nc.dram_tensor("wv", [DIM, HD], CD, kind="ExternalInput")
    wo_e = nc.dram_tensor("wo", [NH * HD, HPC * HD], CD, kind="ExternalInput")
    ch_e = nc.dram_tensor("cosz", [HD, SEQ], CD, kind="ExternalInput")
    sh_e = nc.dram_tensor("sinz", [HD, SEQ], CD, kind="ExternalInput")
    sw_e = nc.dram_tensor("swp", [HD, HD], CD, kind="ExternalInput")
    id_e = nc.dram_tensor("iden", [HD, HD], CD, kind="ExternalInput")
    mk_e = nc.dram_tensor("mask", [HD, HD], F32, kind="ExternalInput")
    out_e = nc.dram_tensor("out", [HPC * HD, SEQ], F32, kind="ExternalOutput")

    agin = [nc.dram_tensor(f"agin{j}", [HPC * HD, 512], CD) for j in range(NCH)]
    agout = [
        nc.dram_tensor(f"agout{j}", [NH * HD, 512], CD, addr_space="Shared")
        for j in range(NCH)
    ]

    with tile.TileContext(nc) as tc:
        _build_body(nc, tc, locals())
    nc.compile()
    return nc


def _build_body(nc, tc, ext):
    from contextlib import ExitStack

    x_e, wq_e, wk_e, wv_e, wo_e = (ext[k] for k in ("x_e", "wq_e", "wk_e", "wv_e", "wo_e"))
    ch_e, sh_e, sw_e, id_e, mk_e = (ext[k] for k in ("ch_e", "sh_e", "sw_e", "id_e", "mk_e"))
    out_e, agin, agout = ext["out_e"], ext["agin"], ext["agout"]

    with ExitStack() as ctx:
        consts = ctx.enter_context(tc.tile_pool(name="consts", bufs=1))
        xin = ctx.enter_context(tc.tile_pool(name="xin", bufs=36))
        qtp = ctx.enter_context(tc.tile_pool(name="qtp", bufs=2))
        rope = ctx.enter_context(tc.tile_pool(name="rope", bufs=2))
        epool = ctx.enter_context(tc.tile_pool(name="epool", bufs=4))
        atp = ctx.enter_context(tc.tile_pool(name="atp", bufs=2))
        small = ctx.enter_context(tc.tile_pool(name="small", bufs=3))
        ocp = ctx.enter_context(tc.tile_pool(name="ocp", bufs=2))
        agp = ctx.enter_context(tc.tile_pool(name="agp", bufs=42))
        # PSUM: 3 + 2 + 2 + 1 = 8 banks
        accp = ctx.enter_context(tc.tile_pool(name="accp", bufs=3, space="PSUM"))
        ps512 = ctx.enter_context(tc.tile_pool(name="ps512", bufs=2, space="PSUM"))
        apsp = ctx.enter_context(tc.tile_pool(name="apsp", bufs=2, space="PSUM"))
        pstr = ctx.enter_context(tc.tile_pool(name="pstr", bufs=1, space="PSUM"))

        # ---- persistent tensors ----
        ch_sb = consts.tile([HD, SEQ], CD, name="ch_sb")
        sh_sb = consts.tile([HD, SEQ], CD, name="sh_sb")
        sw_sb = consts.tile([HD, HD], CD, name="sw_sb")
        id_sb = consts.tile([HD, HD], CD, name="id_sb")
        mk_sb = consts.tile([HD, HD], F32, name="mk_sb")
        wq_sb = consts.tile([128, NDT * HPC * 128], CD, name="wq_sb")
        wk_sb = consts.tile([128, NDT * 128], CD, name="wk_sb")
        wv_sb = consts.tile([128, NDT * 128], CD, name="wv_sb")
        wo_sb = consts.tile([128, NET * 512], CD, name="wo_sb")
        kt_sb = consts.tile([128, SEQ], CD, name="kt_sb")            # [hd, seq]
        v_sb = consts.tile([128, (SEQ // 128) * (HD + 1)], CD, name="v_sb")
        nc.vector.memset(
            v_sb[:].rearrange("p (t c) -> p t c", c=HD + 1)[:, :, HD:HD + 1], 1.0
        )

        # ---- PE clock warmup: the HAM throttles a cold PE to 1.2 GHz for
        # ~3.4us; dummy matmuls during the initial DMA wait un-throttle it
        # before the first real projection matmul ----
        warm = consts.tile([128, 128], CD, name="warm")
        nc.vector.memset(warm[:], 0.0)
        wps = pstr.tile([128, 64], F32, tag="tr", name="warm_ps")
        for _ in range(20):
            nc.tensor.matmul(wps[:], warm[:], warm[:, 0:64], start=True, stop=True)

        # ---- weight + const loads: gpsimd SWDGE queue, large descriptors ----
        # wq in 4 blocks of 8 d-tiles so chunk-0 compute can start early
        wq3 = wq_sb[:].rearrange("p (d c) -> p d c", d=NDT)
        wqe3 = wq_e[:, :].rearrange("(d p) c -> p d c", p=128)
        for b in range(NXB):
            nc.gpsimd.dma_start(
                wq3[:, XB * b:XB * (b + 1), :], wqe3[:, XB * b:XB * (b + 1), :]
            )
        nc.gpsimd.dma_start(
            wk_sb[:].rearrange("p (d c) -> p d c", d=NDT),
            wk_e[:, :].rearrange("(d p) c -> p d c", p=128),
        )
        nc.gpsimd.dma_start(
            wv_sb[:].rearrange("p (d c) -> p d c", d=NDT),
            wv_e[:, :].rearrange("(d p) c -> p d c", p=128),
        )
        nc.gpsimd.dma_start(ch_sb[:], ch_e[:, :])
        nc.gpsimd.dma_start(sh_sb[:], sh_e[:, :])
        nc.gpsimd.dma_start(sw_sb[:], sw_e[:, :])
        nc.gpsimd.dma_start(id_sb[:], id_e[:, :])
        nc.gpsimd.dma_start(mk_sb[:], mk_e[:, :])
        def load_wo():
            # deferred to iteration 1: the 4MB wo load would steal HBM
            # bandwidth from chunk-0/1 x tiles during the startup crunch
            wo3 = wo_sb[:].rearrange("p (e c) -> p e c", e=NET)
            woe3 = wo_e[:, :].rearrange("(e p) c -> p e c", p=128)
            for b in range(2):
                nc.gpsimd.dma_start(
                    wo3[:, 16 * b:16 * (b + 1), :], woe3[:, 16 * b:16 * (b + 1), :]
                )

        def rope_emit(j, m, acc, qt_j):
            """acc: psum [128,512] projection of q head m (m<HPC) or k (m==HPC)."""
            t_sb = rope.tile([128, 512], CD, tag="tsb", name=f"t_{j}_{m}")
            nc.vector.tensor_copy(t_sb[:], acc[:])
            ups = ps512.tile([128, 512], F32, tag="b512", name=f"u_{j}_{m}")
            nc.tensor.matmul(ups[:], sw_sb[:], t_sb[:], start=True, stop=True)
            m1 = rope.tile([128, 512], CD, tag="m1", name=f"m1_{j}_{m}")
            nc.vector.tensor_tensor(
                m1[:], t_sb[:], ch_sb[:, 512 * j:512 * (j + 1)], op=ALU.mult
            )
            if m < HPC:
                dest = qt_j[:, 512 * m:512 * (m + 1)]
            else:
                dest = kt_sb[:, 512 * j:512 * (j + 1)]
            nc.vector.tensor_tensor(
                dest, ups[:], sh_sb[:, 512 * j:512 * (j + 1)], op=ALU.mult
            )
            nc.vector.tensor_add(dest, dest, m1[:])

        def proj_chunk(j, xts, qt_j):
            """q/k/v projections + rope for seq chunk j, in two 3-bank passes."""
            # pass 1: q heads 0,1 + k
            accs = [
                accp.tile([128, 512], F32, tag="acc", name=f"p1a_{j}_{i}")
                for i in range(3)
            ]
            for d in range(NDT):
                xsl = xts[d][:]
                for i, m in enumerate((0, 1)):
                    nc.tensor.matmul(
                        accs[i][:],
                        wq_sb[:, 512 * d + 128 * m:512 * d + 128 * (m + 1)],
                        xsl,
                        start=(d == 0), stop=(d == NDT - 1),
                    )
                nc.tensor.matmul(
                    accs[2][:], wk_sb[:, 128 * d:128 * (d + 1)], xsl,
                    start=(d == 0), stop=(d == NDT - 1),
                )
            rope_emit(j, 0, accs[0], qt_j)
            rope_emit(j, 1, accs[1], qt_j)
            rope_emit(j, HPC, accs[2], qt_j)
            # pass 2: q heads 2,3 + v (weight-stationary, vT [hd, seq])
            accs = [
                accp.tile([128, 512], F32, tag="acc", name=f"p2a_{j}_{i}")
                for i in range(3)
            ]
            for d in range(NDT):
                xsl = xts[d][:]
                for i, m in enumerate((2, 3)):
                    nc.tensor.matmul(
                        accs[i][:],
                        wq_sb[:, 512 * d + 128 * m:512 * d + 128 * (m + 1)],
                        xsl,
                        start=(d == 0), stop=(d == NDT - 1),
                    )
                nc.tensor.matmul(
                    accs[2][:], wv_sb[:, 128 * d:128 * (d + 1)], xsl,
                    start=(d == 0), stop=(d == NDT - 1),
                )
            rope_emit(j, 2, accs[0], qt_j)
            rope_emit(j, 3, accs[1], qt_j)
            # vT [hd, 512] -> transpose into v_sb [seq-part, (t, hd+1)]
            vt_sb = rope.tile([128, 512], CD, tag="vt", name=f"vt_{j}")
            nc.vector.tensor_copy(vt_sb[:], accs[2][:])
            for st in range(4):
                t = 4 * j + st
                trp = pstr.tile([128, 128], CD, tag="tr", name=f"vtr_{j}_{st}")
                nc.tensor.transpose(
                    trp[:], vt_sb[:, 128 * st:128 * (st + 1)], id_sb[:]
                )
                nc.vector.tensor_copy(
                    v_sb[:, (HD + 1) * t:(HD + 1) * t + HD], trp[:]
                )

        def att_chunk(j, qt_j):
            """attention for seq chunk j, 4 heads; writes at_sb, returns it."""
            at_sb = atp.tile([128, HPC * 512], CD, tag="atT", name=f"atT{j}")
            for h in range(HPC):
                qsl = qt_j[:, 512 * h:512 * (h + 1)]
                # 4 sq accumulators packed 2-per-bank: [128, 258] tiles
                apt = [
                    apsp.tile([128, 2 * (HD + 1)], F32, tag="aps", name=f"ap_{j}_{h}_{p}")
                    for p in range(2)
                ]
                aps = [apt[sq // 2][:, (HD + 1) * (sq % 2):(HD + 1) * (sq % 2 + 1)]
                       for sq in range(4)]
                for skt in range(4 * j + 4):
                    r = skt - 4 * j
                    lo = 128 * r if r > 0 else 0
                    stp = ps512.tile([128, 512], F32, tag="b512", name=f"st_{j}_{h}_{skt}")
                    nc.tensor.matmul(
                        stp[:, lo:512],
                        kt_sb[:, 128 * skt:128 * (skt + 1)],
                        qsl[:, lo:512],
                        start=True, stop=True,
                    )
                    E = epool.tile([128, 512], CD, tag="E", name=f"E_{j}_{h}_{skt}")
                    if r >= 0:
                        nc.vector.tensor_add(
                            stp[:, 128 * r:128 * (r + 1)],
                            stp[:, 128 * r:128 * (r + 1)],
                            mk_sb[:],
                        )
                        nc.scalar.activation(E[:, lo:512], stp[:, lo:512], AF.Exp)
                    else:
                        nc.scalar.activation(E[:], stp[:], AF.Exp)
                    for sq in range(max(0, r), 4):
                        # two accumulation groups share each psum bank: only the
                        # bank's first touch may set start (it zeroes the whole
                        # 2KB zero-region) and only its last touch sets stop.
                        nc.tensor.matmul(
                            aps[sq],
                            E[:, 128 * sq:128 * (sq + 1)],
                            v_sb[:, (HD + 1) * skt:(HD + 1) * (skt + 1)],
                            start=(skt == 0 and sq % 2 == 0),
                            stop=(skt == 4 * j + sq and sq % 2 == 1),
                            skip_group_check=True,
                        )
                        if skt == 4 * j + sq:  # this sq-subtile is complete
                            inv = small.tile([128, 1], F32, tag="inv", name=f"i_{j}_{h}_{sq}")
                            nc.vector.reciprocal(inv[:], aps[sq][:, HD:HD + 1])
                            an = small.tile([128, 128], CD, tag="an", name=f"an_{j}_{h}_{sq}")
                            nc.vector.tensor_scalar_mul(
                                an[:], aps[sq][:, 0:HD], inv[:]
                            )
                            trp = pstr.tile([128, 128], CD, tag="tr", name=f"tr_{j}_{h}_{sq}")
                            nc.tensor.transpose(trp[:], an[:], id_sb[:])
                            nc.vector.tensor_copy(
                                at_sb[:, 512 * h + 128 * sq:512 * h + 128 * (sq + 1)],
                                trp[:],
                            )
            return at_sb

        def pull_ag(j, eng, eng2=None):
            """agout[j] -> SBUF tiles, one DMA per 128-row et tile.  With
            eng2, even/odd tiles alternate queues (2x arrival rate)."""
            agts = []
            for et in range(NET):
                agt = agp.tile([128, 512], CD, tag="agt", name=f"ag_{j}_{et}")
                e = eng2 if (eng2 is not None and et % 2 == 1) else eng
                e.dma_start(agt[:], agout[j][128 * et:128 * (et + 1), :])
                agts.append(agt)
            return agts

        def emit_wo(j, agts, single=False):
            """out projection for seq chunk j.

            Transposed form: stationary = wo_sb slice (resident weight tile,
            same pattern as the q/k/v projections), moving = agt tile.  Output
            is outT [oc, seq]; the host transposes back.

            In-loop: two 2-outcol-block passes (3 accp banks, attention keeps
            ps512).  Epilogue (single=True): one 4-block pass borrowing a
            ps512 bank -- each agt tile read once, so the pulls keep pace."""
            passes = [(0, 1, 2, 3)] if single else [(0, 1), (2, 3)]
            for p, blks in enumerate(passes):
                wops = [
                    accp.tile([128, 512], F32, tag="acc", name=f"wop_{j}_{p}_{i}")
                    for i in range(min(len(blks), 3))
                ]
                if len(blks) == 4:
                    wops.append(
                        ps512.tile([128, 512], F32, tag="b512", name=f"wop_{j}_{p}_3")
                    )
                for et in range(NET):
                    for i, bb in enumerate(blks):
                        nc.tensor.matmul(
                            wops[i][:],
                            wo_sb[:, 512 * et + 128 * bb:512 * et + 128 * (bb + 1)],
                            agts[et][:],
                            start=(et == 0),
                            stop=(et == NET - 1),
                        )
                for i, bb in enumerate(blks):
                    oc = ocp.tile([128, 512], F32, tag="oc", name=f"oc_{j}_{bb}")
                    nc.vector.tensor_copy(oc[:], wops[i][:])
                    # gpsimd, NOT sync: a sync-queue write would head-of-line
                    # block the next chunk's x prefetch behind the wo drain
                    nc.gpsimd.dma_start(
                        out_e[128 * bb:128 * (bb + 1), 512 * j:512 * (j + 1)], oc[:]
                    )

        # ================= fused main loop =================
        agts_pend = {}
        for j in range(NCH):
            if j >= 2:
                # gpsimd, before x(j): on sync their pool-slot waits would
                # head-of-line block x(j+1).  j-2 (not j-1): the scheduler
                # models collectives as fast and statically interleaves the
                # consuming wo early; with j-1 those matmuls stall the
                # in-order PE queue on the still-running AllGather.
                agts_pend[j - 2] = pull_ag(j - 2, nc.gpsimd)
            # x chunk j: per-d-tile 2D DMAs on the sync queue (contiguous
            # 128-row slices stream at full HBM rate; 3D-pattern block loads
            # measured 3x slower)
            xts = []
            for dd in range(NDT):
                xt = xin.tile([128, 512], CD, tag="xin", name=f"x_{j}_{dd}")
                # chunk 0 is arrival-rate-limited: split across both HWDGE
                # rings (scalar ring is idle before the first exps)
                eng = nc.scalar if (j == 0 and dd % 2 == 1) else nc.sync
                eng.dma_start(
                    xt[:], x_e[128 * dd:128 * (dd + 1), 512 * j:512 * (j + 1)]
                )
                xts.append(xt)
            if j == 1:
                load_wo()
            qt_j = qtp.tile([128, HPC * 512], CD, tag="qt", name=f"qt_{j}")
            proj_chunk(j, xts, qt_j)
            at_sb = att_chunk(j, qt_j)
            # AllGather chunk j (agin write on gpsimd: on sync it would
            # head-of-line block the next chunk's x prefetch behind att(j))
            nc.gpsimd.dma_start(
                agin[j][:, :].rearrange("(h p) s -> p h s", h=HPC),
                at_sb[:].rearrange("p (h s) -> p h s", h=HPC),
            )
            nc.gpsimd.collective_compute(
                "AllGather",
                ALU.bypass,
                replica_groups=[list(range(NCORE))],
                ins=[agin[j][:, :]],
                outs=[agout[j][:, :]],
            )
            if j >= 2:
                emit_wo(j - 2, agts_pend.pop(j - 2))
        # epilogue: both pulls emitted before the wo's so the last chunks'
        # agout DMAs stream during wo(NCH-2)'s matmuls; pulls alternate
        # queues so arrival outpaces consumption
        agts_a = pull_ag(NCH - 2, nc.sync, nc.gpsimd)
        agts_b = pull_ag(NCH - 1, nc.sync, nc.gpsimd)
        emit_wo(NCH - 2, agts_a)
        emit_wo(NCH - 1, agts_b)


# ---------------- host side ----------------
_PERM = np.concatenate([np.arange(0, HD, 2), np.arange(1, HD, 2)])
_NC_CACHE = {}


def _get_nc():
    if "nc" not in _NC_CACHE:
        _NC_CACHE["nc"] = build()
    return _NC_CACHE["nc"]


def _prep_consts():
    freqs = 1.0 / (THETA ** (np.arange(HALF, dtype=np.float64) / HALF))
    ang = np.arange(SEQ, dtype=np.float64)[:, None] * freqs[None, :]
    cos = np.cos(ang).astype(np.float32)
    sin = np.sin(ang).astype(np.float32)
    CH = np.ascontiguousarray(np.concatenate([cos, cos], axis=1).T)
    SH = np.ascontiguousarray(np.concatenate([-sin, sin], axis=1).T)
    S_l = np.zeros((HD, HD), np.float32)
    for i in range(HD):
        S_l[(i + 64) % HD, i] = 1.0
    iden = np.eye(HD, dtype=np.float32)
    mask = np.where(
        np.arange(HD)[:, None] <= np.arange(HD)[None, :], 0.0, -1e30
    ).astype(np.float32)
    return CH, SH, S_l, iden, mask


def _cd(a):
    import ml_dtypes
    return np.ascontiguousarray(a).astype(ml_dtypes.bfloat16)


def kernel(x, wq, wk, wv, wo):
    x, wq, wk, wv, wo = (np.asarray(a, dtype=np.float32) for a in (x, wq, wk, wv, wo))
    nc = _get_nc()
    CH, SH, S_l, iden, mask = _prep_consts()
    xT = np.ascontiguousarray(x.T)
    wq_p = wq.reshape(NH, HD, DIM)[:, _PERM, :] * SCALE
    wk_p = wk.reshape(NKV, HD, DIM)[:, _PERM, :]
    xT_c = _cd(xT)
    CH_c, SH_c, S_c, id_c = _cd(CH), _cd(SH), _cd(S_l), _cd(iden)
    in_maps = []
    for c in range(NCORE):
        in_maps.append(
            {
                "x": xT_c,
                "wq": _cd(wq_p[HPC * c:HPC * (c + 1)].reshape(HPC * HD, DIM).T),
                "wk": _cd(wk_p[c].T),
                "wv": _cd(wv[HD * c:HD * (c + 1), :].T),
                "wo": _cd(wo[HPC * HD * c:HPC * HD * (c + 1), :].T),
                "cosz": CH_c,
                "sinz": SH_c,
                "swp": S_c,
                "iden": id_c,
                "mask": mask,
            }
        )
    res = run_bass_kernel_spmd(nc, in_maps, core_ids=list(range(NCORE)))
    out = np.concatenate([res.results[c]["out"].T for c in range(NCORE)], axis=1)
    return np.ascontiguousarray(out, dtype=np.float32)


# revision 41
# speedup vs baseline: 1.0315x; 1.0315x over previous
"""Distributed GQA attention layer (seq=2048, dim=4096, 32 q heads / 8 kv heads,
rope theta=5e5, causal) on 8 TRN2 NeuronCores.

Sharding: tensor-parallel over heads. Core c owns q heads 4c..4c+3 and kv head c.
Each core computes its 4 heads' attention output in transposed layout
attnT_local [512, 2048], an AllGather over the partition axis assembles
attnT_full [4096, 2048], and each core then computes 512 output columns of the
final projection: out_c [2048, 512] = attnT_full.T @ woT_c.  The host
concatenates the 8 column blocks.

v2 structure: one fused loop over the 4 seq chunks -- projections(j),
attention(j), AllGather(j), out-projection(j-2) -- so collectives start as
early as possible and the out-projection pipeline hides them.  DMA traffic is
split between the sync HWDGE queue (x tiles, agin) and the gpsimd SWDGE queue
(weights, agout pulls, output writes) with large descriptors.

Device-side layout tricks (all host-prepped):
 - x fed transposed [dim, seq]; all weights fed as [dim(contract), out].
 - rope pairs (2i, 2i+1) are permuted to half-split form by permuting wq/wk
   rows, making rope a "rotate-half": r = t*CH + swap_halves(t)*SH, where
   swap_halves is a 128x128 permutation matmul and CH/SH are [128, seq]
   tables. The q-k inner product is invariant under the shared permutation.
 - 1/sqrt(hd) folded into wq.
 - softmax runs unnormalized; a ones-column appended to v makes the PV matmul
   accumulate the denominator in psum column 128, and the normalization is
   fused into the per-q-row scale before the transpose.
 - v projected weight-stationary into [hd, seq] then PE-transposed to
   [seq, hd] tiles.
PSUM budget (8 banks): accp 3 (q-projection 2-head passes + k/v acc; also the
wo accumulators in 2-sq passes), ps512 2 (rope swap product + score tiles),
apsp 2 (PV accumulators, 2x129 cols packed per bank), pstr 1 (transposes).
"""
import sys

sys.path.insert(0, "/opt/trn_rl_repo")
import numpy as np

import concourse.bass as bass
import concourse.mybir as mybir
import concourse.tile as tile
from concourse import bacc
from concourse.bass_utils import run_bass_kernel_spmd

SEQ = 2048
DIM = 4096
NH, NKV, HD = 32, 8, 128
THETA = 500000.0
NCORE = 8
HPC = NH // NCORE          # 4 q heads per core
HALF = HD // 2
SCALE = 1.0 / np.sqrt(HD)
NDT = DIM // 128           # contraction tiles for qkv projections (32)
NET = (NH * HD) // 128     # e-dim tiles for the output projection (32)
NCH = SEQ // 512           # 4 seq chunks
XB = 8                     # x d-tiles per DMA block
NXB = NDT // XB            # 4 x-blocks per chunk
F32 = mybir.dt.float32
AF = mybir.ActivationFunctionType
ALU = mybir.AluOpType
CD = mybir.dt.bfloat16


def build():
    nc = bacc.Bacc("TRN2", target_bir_lowering=False, debug=False, num_devices=NCORE)
    x_e = nc.dram_tensor("x", [DIM, SEQ], CD, kind="ExternalInput")
    wq_e = nc.dram_tensor("wq", [DIM, HPC * HD], CD, kind="ExternalInput")
    wk_e = nc.dram_tensor("wk", [DIM, HD], CD, kind="ExternalInput")
    wv_e = nc.dram_tensor("wv", [DIM, HD], CD, kind="ExternalInput")
    wo_e = nc.dram_tensor("wo", [NH * HD, HPC * HD], CD, kind="ExternalInput")
    ch_e = nc.dram_tensor("cosz", [HD, SEQ], CD, kind="ExternalInput")
    sh_e = nc.dram_tensor("sinz", [HD, SEQ], CD, kind="ExternalInput")
    sw_e = nc.dram_tensor("swp", [HD, HD], CD, kind="ExternalInput")
    id_e = nc.dram_tensor("iden", [HD, HD], CD, kind="ExternalInput")
    mk_e = nc.dram_tensor("mask", [HD, HD], F32, kind="ExternalInput")
    out_e = nc.dram_tensor("out", [HPC * HD, SEQ], F32, kind="ExternalOutput")

    agin = [nc.dram_tensor(f"agin{j}", [HPC * HD, 512], CD) for j in range(NCH)]
    agout = [
        nc.dram_tensor(f"agout{j}", [NH * HD, 512], CD, addr_space="Shared")
        for j in range(NCH)
    ]

    with tile.TileContext(nc) as tc:
        _build_body(nc, tc, locals())
    nc.compile()
    return nc


def _build_body(nc, tc, ext):
    from contextlib import ExitStack

    x_e, wq_e, wk_e, wv_e, wo_e = (ext[k] for k in ("x_e", "wq_e", "wk_e", "wv_e", "wo_e"))
    ch_e, sh_e, sw_e, id_e, mk_e = (ext[k] for k in ("ch_e", "sh_e", "sw_e", "id_e", "mk_e"))
    out_e, agin, agout = ext["out_e"], ext["agin"], ext["agout"]

    with ExitStack() as ctx:
        consts = ctx.enter_context(tc.tile_pool(name="consts", bufs=1))
        xin = ctx.enter_context(tc.tile_pool(name="xin", bufs=40))
        qtp = ctx.enter_context(tc.tile_pool(name="qtp", bufs=2))
        rope = ctx.enter_context(tc.tile_pool(name="rope", bufs=2))
        epool = ctx.enter_context(tc.tile_pool(name="epool", bufs=4))
        atp = ctx.enter_context(tc.tile_pool(name="atp", bufs=2))
        small = ctx.enter_context(tc.tile_pool(name="small", bufs=3))
        ocp = ctx.enter_context(tc.tile_pool(name="ocp", bufs=2))
        agp = ctx.enter_context(tc.tile_pool(name="agp", bufs=38))
        # PSUM: 3 + 2 + 2 + 1 = 8 banks
        accp = ctx.enter_context(tc.tile_pool(name="accp", bufs=3, space="PSUM"))
        ps512 = ctx.enter_context(tc.tile_pool(name="ps512", bufs=2, space="PSUM"))
        apsp = ctx.enter_context(tc.tile_pool(name="apsp", bufs=2, space="PSUM"))
        pstr = ctx.enter_context(tc.tile_pool(name="pstr", bufs=1, space="PSUM"))

        # ---- persistent tensors ----
        ch_sb = consts.tile([HD, SEQ], CD, name="ch_sb")
        sh_sb = consts.tile([HD, SEQ], CD, name="sh_sb")
        sw_sb = consts.tile([HD, HD], CD, name="sw_sb")
        id_sb = consts.tile([HD, HD], CD, name="id_sb")
        mk_sb = consts.tile([HD, HD], F32, name="mk_sb")
        wq_sb = consts.tile([128, NDT * HPC * 128], CD, name="wq_sb")
        wk_sb = consts.tile([128, NDT * 128], CD, name="wk_sb")
        wv_sb = consts.tile([128, NDT * 128], CD, name="wv_sb")
        wo_sb = consts.tile([128, NET * 512], CD, name="wo_sb")
        kt_sb = consts.tile([128, SEQ], CD, name="kt_sb")            # [hd, seq]
        v_sb = consts.tile([128, (SEQ // 128) * (HD + 1)], CD, name="v_sb")
        nc.vector.memset(
            v_sb[:].rearrange("p (t c) -> p t c", c=HD + 1)[:, :, HD:HD + 1], 1.0
        )

        # ---- weight + const loads: gpsimd SWDGE queue, large descriptors ----
        # wq in 4 blocks of 8 d-tiles so chunk-0 compute can start early
        wq3 = wq_sb[:].rearrange("p (d c) -> p d c", d=NDT)
        wqe3 = wq_e[:, :].rearrange("(d p) c -> p d c", p=128)
        for b in range(NXB):
            nc.gpsimd.dma_start(
                wq3[:, XB * b:XB * (b + 1), :], wqe3[:, XB * b:XB * (b + 1), :]
            )
        nc.gpsimd.dma_start(
            wk_sb[:].rearrange("p (d c) -> p d c", d=NDT),
            wk_e[:, :].rearrange("(d p) c -> p d c", p=128),
        )
        nc.gpsimd.dma_start(
            wv_sb[:].rearrange("p (d c) -> p d c", d=NDT),
            wv_e[:, :].rearrange("(d p) c -> p d c", p=128),
        )
        nc.gpsimd.dma_start(ch_sb[:], ch_e[:, :])
        nc.gpsimd.dma_start(sh_sb[:], sh_e[:, :])
        nc.gpsimd.dma_start(sw_sb[:], sw_e[:, :])
        nc.gpsimd.dma_start(id_sb[:], id_e[:, :])
        nc.gpsimd.dma_start(mk_sb[:], mk_e[:, :])
        def load_wo():
            # deferred to iteration 1: the 4MB wo load would steal HBM
            # bandwidth from chunk-0/1 x tiles during the startup crunch
            wo3 = wo_sb[:].rearrange("p (e c) -> p e c", e=NET)
            woe3 = wo_e[:, :].rearrange("(e p) c -> p e c", p=128)
            for b in range(2):
                nc.gpsimd.dma_start(
                    wo3[:, 16 * b:16 * (b + 1), :], woe3[:, 16 * b:16 * (b + 1), :]
                )

        def rope_emit(j, m, acc, qt_j):
            """acc: psum [128,512] projection of q head m (m<HPC) or k (m==HPC)."""
            t_sb = rope.tile([128, 512], CD, tag="tsb", name=f"t_{j}_{m}")
            nc.vector.tensor_copy(t_sb[:], acc[:])
            ups = ps512.tile([128, 512], F32, tag="b512", name=f"u_{j}_{m}")
            nc.tensor.matmul(ups[:], sw_sb[:], t_sb[:], start=True, stop=True)
            m1 = rope.tile([128, 512], CD, tag="m1", name=f"m1_{j}_{m}")
            nc.vector.tensor_tensor(
                m1[:], t_sb[:], ch_sb[:, 512 * j:512 * (j + 1)], op=ALU.mult
            )
            if m < HPC:
                dest = qt_j[:, 512 * m:512 * (m + 1)]
            else:
                dest = kt_sb[:, 512 * j:512 * (j + 1)]
            nc.vector.tensor_tensor(
                dest, ups[:], sh_sb[:, 512 * j:512 * (j + 1)], op=ALU.mult
            )
            nc.vector.tensor_add(dest, dest, m1[:])

        def proj_chunk(j, xts, qt_j):
            """q/k/v projections + rope for seq chunk j, in two 3-bank passes."""
            # pass 1: q heads 0,1 + k
            accs = [
                accp.tile([128, 512], F32, tag="acc", name=f"p1a_{j}_{i}")
                for i in range(3)
            ]
            for d in range(NDT):
                xsl = xts[d][:]
                for i, m in enumerate((0, 1)):
                    nc.tensor.matmul(
                        accs[i][:],
                        wq_sb[:, 512 * d + 128 * m:512 * d + 128 * (m + 1)],
                        xsl,
                        start=(d == 0), stop=(d == NDT - 1),
                    )
                nc.tensor.matmul(
                    accs[2][:], wk_sb[:, 128 * d:128 * (d + 1)], xsl,
                    start=(d == 0), stop=(d == NDT - 1),
                )
            rope_emit(j, 0, accs[0], qt_j)
            rope_emit(j, 1, accs[1], qt_j)
            rope_emit(j, HPC, accs[2], qt_j)
            # pass 2: q heads 2,3 + v (weight-stationary, vT [hd, seq])
            accs = [
                accp.tile([128, 512], F32, tag="acc", name=f"p2a_{j}_{i}")
                for i in range(3)
            ]
            for d in range(NDT):
                xsl = xts[d][:]
                for i, m in enumerate((2, 3)):
                    nc.tensor.matmul(
                        accs[i][:],
                        wq_sb[:, 512 * d + 128 * m:512 * d + 128 * (m + 1)],
                        xsl,
                        start=(d == 0), stop=(d == NDT - 1),
                    )
                nc.tensor.matmul(
                    accs[2][:], wv_sb[:, 128 * d:128 * (d + 1)], xsl,
                    start=(d == 0), stop=(d == NDT - 1),
                )
            rope_emit(j, 2, accs[0], qt_j)
            rope_emit(j, 3, accs[1], qt_j)
            # vT [hd, 512] -> transpose into v_sb [seq-part, (t, hd+1)]
            vt_sb = rope.tile([128, 512], CD, tag="vt", name=f"vt_{j}")
            nc.vector.tensor_copy(vt_sb[:], accs[2][:])
            for st in range(4):
                t = 4 * j + st
                trp = pstr.tile([128, 128], CD, tag="tr", name=f"vtr_{j}_{st}")
                nc.tensor.transpose(
                    trp[:], vt_sb[:, 128 * st:128 * (st + 1)], id_sb[:]
                )
                nc.vector.tensor_copy(
                    v_sb[:, (HD + 1) * t:(HD + 1) * t + HD], trp[:]
                )

        def att_chunk(j, qt_j):
            """attention for seq chunk j, 4 heads; writes at_sb, returns it."""
            at_sb = atp.tile([128, HPC * 512], CD, tag="atT", name=f"atT{j}")
            for h in range(HPC):
                qsl = qt_j[:, 512 * h:512 * (h + 1)]
                # 4 sq accumulators packed 2-per-bank: [128, 258] tiles
                apt = [
                    apsp.tile([128, 2 * (HD + 1)], F32, tag="aps", name=f"ap_{j}_{h}_{p}")
                    for p in range(2)
                ]
                aps = [apt[sq // 2][:, (HD + 1) * (sq % 2):(HD + 1) * (sq % 2 + 1)]
                       for sq in range(4)]
                for skt in range(4 * j + 4):
                    r = skt - 4 * j
                    lo = 128 * r if r > 0 else 0
                    stp = ps512.tile([128, 512], F32, tag="b512", name=f"st_{j}_{h}_{skt}")
                    nc.tensor.matmul(
                        stp[:, lo:512],
                        kt_sb[:, 128 * skt:128 * (skt + 1)],
                        qsl[:, lo:512],
                        start=True, stop=True,
                    )
                    E = epool.tile([128, 512], CD, tag="E", name=f"E_{j}_{h}_{skt}")
                    if r >= 0:
                        nc.vector.tensor_add(
                            stp[:, 128 * r:128 * (r + 1)],
                            stp[:, 128 * r:128 * (r + 1)],
                            mk_sb[:],
                        )
                        nc.scalar.activation(E[:, lo:512], stp[:, lo:512], AF.Exp)
                    else:
                        nc.scalar.activation(E[:], stp[:], AF.Exp)
                    for sq in range(max(0, r), 4):
                        # two accumulation groups share each psum bank: only the
                        # bank's first touch may set start (it zeroes the whole
                        # 2KB zero-region) and only its last touch sets stop.
                        nc.tensor.matmul(
                            aps[sq],
                            E[:, 128 * sq:128 * (sq + 1)],
                            v_sb[:, (HD + 1) * skt:(HD + 1) * (skt + 1)],
                            start=(skt == 0 and sq % 2 == 0),
                            stop=(skt == 4 * j + sq and sq % 2 == 1),
                            skip_group_check=True,
                        )
                        if skt == 4 * j + sq:  # this sq-subtile is complete
                            inv = small.tile([128, 1], F32, tag="inv", name=f"i_{j}_{h}_{sq}")
                            nc.vector.reciprocal(inv[:], aps[sq][:, HD:HD + 1])
                            an = small.tile([128, 128], CD, tag="an", name=f"an_{j}_{h}_{sq}")
                            nc.vector.tensor_scalar_mul(
                                an[:], aps[sq][:, 0:HD], inv[:]
                            )
                            trp = pstr.tile([128, 128], CD, tag="tr", name=f"tr_{j}_{h}_{sq}")
                            nc.tensor.transpose(trp[:], an[:], id_sb[:])
                            nc.vector.tensor_copy(
                                at_sb[:, 512 * h + 128 * sq:512 * h + 128 * (sq + 1)],
                                trp[:],
                            )
            return at_sb

        def pull_ag(j, eng, eng2=None):
            """agout[j] -> SBUF tiles, one DMA per 128-row et tile.  With
            eng2, even/odd tiles alternate queues (2x arrival rate)."""
            agts = []
            for et in range(NET):
                agt = agp.tile([128, 512], CD, tag="agt", name=f"ag_{j}_{et}")
                e = eng2 if (eng2 is not None and et % 2 == 1) else eng
                e.dma_start(agt[:], agout[j][128 * et:128 * (et + 1), :])
                agts.append(agt)
            return agts

        def emit_wo(j, agts, single=False):
            """out projection for seq chunk j.

            Transposed form: stationary = wo_sb slice (resident weight tile,
            same pattern as the q/k/v projections), moving = agt tile.  Output
            is outT [oc, seq]; the host transposes back.

            In-loop: two 2-outcol-block passes (3 accp banks, attention keeps
            ps512).  Epilogue (single=True): one 4-block pass borrowing a
            ps512 bank -- each agt tile read once, so the pulls keep pace."""
            passes = [(0, 1, 2, 3)] if single else [(0, 1), (2, 3)]
            for p, blks in enumerate(passes):
                wops = [
                    accp.tile([128, 512], F32, tag="acc", name=f"wop_{j}_{p}_{i}")
                    for i in range(min(len(blks), 3))
                ]
                if len(blks) == 4:
                    wops.append(
                        ps512.tile([128, 512], F32, tag="b512", name=f"wop_{j}_{p}_3")
                    )
                for et in range(NET):
                    for i, bb in enumerate(blks):
                        nc.tensor.matmul(
                            wops[i][:],
                            wo_sb[:, 512 * et + 128 * bb:512 * et + 128 * (bb + 1)],
                            agts[et][:],
                            start=(et == 0),
                            stop=(et == NET - 1),
                        )
                for i, bb in enumerate(blks):
                    oc = ocp.tile([128, 512], F32, tag="oc", name=f"oc_{j}_{bb}")
                    nc.vector.tensor_copy(oc[:], wops[i][:])
                    # gpsimd, NOT sync: a sync-queue write would head-of-line
                    # block the next chunk's x prefetch behind the wo drain
                    nc.gpsimd.dma_start(
                        out_e[128 * bb:128 * (bb + 1), 512 * j:512 * (j + 1)], oc[:]
                    )

        # ================= fused main loop =================
        agts_pend = {}
        for j in range(NCH):
            if j >= 2:
                # gpsimd, before x(j): on sync their pool-slot waits would
                # head-of-line block x(j+1).  j-2 (not j-1): the scheduler
                # models collectives as fast and statically interleaves the
                # consuming wo early; with j-1 those matmuls stall the
                # in-order PE queue on the still-running AllGather.
                agts_pend[j - 2] = pull_ag(j - 2, nc.gpsimd)
            # x chunk j: per-d-tile 2D DMAs on the sync queue (contiguous
            # 128-row slices stream at full HBM rate; 3D-pattern block loads
            # measured 3x slower)
            xts = []
            for dd in range(NDT):
                xt = xin.tile([128, 512], CD, tag="xin", name=f"x_{j}_{dd}")
                # chunk 0 is arrival-rate-limited: split across both HWDGE
                # rings (scalar ring is idle before the first exps)
                eng = nc.scalar if (j == 0 and dd % 2 == 1) else nc.sync
                eng.dma_start(
                    xt[:], x_e[128 * dd:128 * (dd + 1), 512 * j:512 * (j + 1)]
                )
                xts.append(xt)
            if j == 1:
                load_wo()
            qt_j = qtp.tile([128, HPC * 512], CD, tag="qt", name=f"qt_{j}")
            proj_chunk(j, xts, qt_j)
            at_sb = att_chunk(j, qt_j)
            # AllGather chunk j (agin write on gpsimd: on sync it would
            # head-of-line block the next chunk's x prefetch behind att(j))
            nc.gpsimd.dma_start(
                agin[j][:, :].rearrange("(h p) s -> p h s", h=HPC),
                at_sb[:].rearrange("p (h s) -> p h s", h=HPC),
            )
            nc.gpsimd.collective_compute(
                "AllGather",
                ALU.bypass,
                replica_groups=[list(range(NCORE))],
                ins=[agin[j][:, :]],
                outs=[agout[j][:, :]],
            )
            if j >= 2:
                emit_wo(j - 2, agts_pend.pop(j - 2))
        # epilogue: both pulls emitted before the wo's so the last chunks'
        # agout DMAs stream during wo(NCH-2)'s matmuls; pulls alternate
        # queues so arrival outpaces consumption
        agts_a = pull_ag(NCH - 2, nc.sync, nc.gpsimd)
        agts_b = pull_ag(NCH - 1, nc.sync, nc.gpsimd)
        emit_wo(NCH - 2, agts_a)
        emit_wo(NCH - 1, agts_b)


# ---------------- host side ----------------
_PERM = np.concatenate([np.arange(0, HD, 2), np.arange(1, HD, 2)])
_NC_CACHE = {}


def _get_nc():
    if "nc" not in _NC_CACHE:
        _NC_CACHE["nc"] = build()
    return _NC_CACHE["nc"]


def _prep_consts():
    freqs = 1.0 / (THETA ** (np.arange(HALF, dtype=np.float64) / HALF))
    ang = np.arange(SEQ, dtype=np.float64)[:, None] * freqs[None, :]
    cos = np.cos(ang).astype(np.float32)
    sin = np.sin(ang).astype(np.float32)
    CH = np.ascontiguousarray(np.concatenate([cos, cos], axis=1).T)
    SH = np.ascontiguousarray(np.concatenate([-sin, sin], axis=1).T)
    S_l = np.zeros((HD, HD), np.float32)
    for i in range(HD):
        S_l[(i + 64) % HD, i] = 1.0
    iden = np.eye(HD, dtype=np.float32)
    mask = np.where(
        np.arange(HD)[:, None] <= np.arange(HD)[None, :], 0.0, -1e30
    ).astype(np.float32)
    return CH, SH, S_l, iden, mask


def _cd(a):
    import ml_dtypes
    return np.ascontiguousarray(a).astype(ml_dtypes.bfloat16)


def kernel(x, wq, wk, wv, wo):
    x, wq, wk, wv, wo = (np.asarray(a, dtype=np.float32) for a in (x, wq, wk, wv, wo))
    nc = _get_nc()
    CH, SH, S_l, iden, mask = _prep_consts()
    xT = np.ascontiguousarray(x.T)
    wq_p = wq.reshape(NH, HD, DIM)[:, _PERM, :] * SCALE
    wk_p = wk.reshape(NKV, HD, DIM)[:, _PERM, :]
    xT_c = _cd(xT)
    CH_c, SH_c, S_c, id_c = _cd(CH), _cd(SH), _cd(S_l), _cd(iden)
    in_maps = []
    for c in range(NCORE):
        in_maps.append(
            {
                "x": xT_c,
                "wq": _cd(wq_p[HPC * c:HPC * (c + 1)].reshape(HPC * HD, DIM).T),
                "wk": _cd(wk_p[c].T),
                "wv": _cd(wv[HD * c:HD * (c + 1), :].T),
                "wo": _cd(wo[HPC * HD * c:HPC * HD * (c + 1), :].T),
                "cosz": CH_c,
                "sinz": SH_c,
                "swp": S_c,
                "iden": id_c,
                "mask": mask,
            }
        )
    res = run_bass_kernel_spmd(nc, in_maps, core_ids=list(range(NCORE)))
    out = np.concatenate([res.results[c]["out"].T for c in range(NCORE)], axis=1)
    return np.ascontiguousarray(out, dtype=np.float32)
